# revision 1
# baseline (speedup 1.0000x reference)
"""Trainium2 Bass kernel for nn_CP_Based (CP-decomposition feature-product layer).

Math: out[b,u] = sum_r prod_f ( x0[b,f]*K[0,r,f,u] + x1[b,f]*K[1,r,f,u] )
  with x0 = 1/sqrt(1+X^2), x1 = X/sqrt(1+X^2).
Factor the normalization out of the f-product:
  out[b,u] = S[b] * sum_r prod_f ( K0[f,ru] + X[b,f]*K1[f,ru] ),
  S[b] = 1/sqrt(prod_f (1+X[b,f]^2)).
The 32-feature product is decomposed into 8 groups of 4 features. Each group's
product is a linear map from the 16 multilinear monomials of its 4 features:
  G_g[b,ru] = sum_m Q_g[b,m] * C_g[m,ru]        (K=32 matmul on TensorE)
with C_g packed on the host from `kernel` (tiny; zero rows pad each group to
32 so every matmul slice is 32-partition aligned). Monomials Q are built
batched for 512 rows at a time on VectorE, transposed via TensorE into wide
PSUM tiles so the monomial index lands on the contraction axis, copied once
per macro to SBUF (ScalarE), then 8 matmuls produce G_g and a 7-multiply
elementwise chain forms prod_g G_g; an indicator matmul sums over rank.

Sharding: pure data-parallel over batch: 131072 rows -> 8 cores x 16384.
"""

import sys

import numpy as np

sys.path.insert(0, "/opt/trn_rl_repo")

import concourse.bacc as bacc  # noqa: E402
import concourse.mybir as mybir  # noqa: E402
from concourse.bass_utils import run_bass_kernel_spmd  # noqa: E402
from concourse.tile import TileContext  # noqa: E402

F32 = mybir.dt.float32
AF = mybir.ActivationFunctionType
OP = mybir.AluOpType
AX = mybir.AxisListType

B_FULL = 131072
N_CORES = 8
B_CORE = B_FULL // N_CORES  # 16384
F = 32
R, U = 10, 8
RU = R * U  # 80
NG = 8  # feature groups of 4
TILE_B = 128
CHUNK = 4  # b-subtiles per macro tile -> N=512 matmuls
MACRO_B = TILE_B * CHUNK  # 512
N_MACRO = B_CORE // MACRO_B  # 32
CG = CHUNK * NG  # 32 (chunk, group) pairs


def build_nc():
    nc = bacc.Bacc()
    # host pre-arranges X as [macro, partition, chunk, feature] so each
    # macro's load is one contiguous 64 KB DMA
    X = nc.dram_tensor(
        "X", [N_MACRO, TILE_B, CHUNK, F], F32, kind="ExternalInput"
    )
    C = nc.dram_tensor("C", [128, 2 * RU], F32, kind="ExternalInput")
    ident = nc.dram_tensor("ident", [128, 128], F32, kind="ExternalInput")
    rind = nc.dram_tensor("rind", [RU, U], F32, kind="ExternalInput")
    out = nc.dram_tensor(
        "out", [N_MACRO, U, MACRO_B], F32, kind="ExternalOutput"
    )

    with TileContext(nc) as tc:
        with (
            tc.tile_pool(name="const", bufs=1) as cpool,
            tc.tile_pool(name="xin", bufs=3) as xpool,
            tc.tile_pool(name="work", bufs=3) as wpool,
            tc.tile_pool(name="qts", bufs=4) as qpool,
            tc.tile_pool(name="ps_t", bufs=2, space="PSUM") as tps,
            tc.tile_pool(name="ps_g", bufs=1, space="PSUM") as gps,
            tc.tile_pool(name="ps_o", bufs=2, space="PSUM") as ops_,
        ):
            c_sb = [
                cpool.tile([64, 2 * RU], F32, tag=f"c{h}", name=f"c{h}")
                for h in range(2)
            ]
            id_sb = cpool.tile([128, 128], F32, tag="id")
            ri_sb = cpool.tile([RU, U], F32, tag="ri")
            for h in range(2):
                nc.sync.dma_start(out=c_sb[h][:], in_=C[64 * h : 64 * (h + 1), :])
            nc.sync.dma_start(out=id_sb[:], in_=ident[:, :])
            nc.sync.dma_start(out=ri_sb[:], in_=rind[:, :])

            for mi in range(N_MACRO):
                b0 = mi * MACRO_B
                # x for 4 chunks: [128 b, 4 c, 32 f]
                xm = xpool.tile([TILE_B, CHUNK, F], F32, tag="x")
                nc.gpsimd.dma_start(out=xm[:], in_=X[mi])

                # --- S = 1/sqrt(prod_f (1+x^2)) for all 4 chunks ---
                sq = wpool.tile([TILE_B, CHUNK, F], F32, tag="sq")
                s_p = wpool.tile([TILE_B, CHUNK], F32, tag="s_p")
                s_r = wpool.tile([TILE_B, CHUNK], F32, tag="s_r")
                s_t = wpool.tile([TILE_B, CHUNK], F32, tag="s_t")
                nc.vector.tensor_mul(sq[:], xm[:], xm[:])
                nc.vector.tensor_scalar_add(sq[:], sq[:], 1.0)
                nc.vector.tensor_reduce(s_p[:], sq[:], AX.X, OP.mult)
                nc.vector.reciprocal(s_r[:], s_p[:])
                nc.scalar.sqrt(s_t[:], s_r[:])

                # --- monomial halves, batched over (chunk, group) = cg ---
                # pab[128, cg, 4] = (1, Xa, Xb, XaXb); pcd[128, cg, 4]
                pab = wpool.tile([TILE_B, CG, 4], F32, tag="pab")
                pcd = wpool.tile([TILE_B, CG, 4], F32, tag="pcd")
                xg = xm[:].rearrange("p c (g j) -> p (c g) j", j=4)
                nc.vector.memset(pab[:, :, 0:1], 1.0)
                nc.vector.memset(pcd[:, :, 0:1], 1.0)
                nc.vector.tensor_copy(pab[:, :, 1:3], xg[:, :, 0:2])
                nc.vector.tensor_copy(pcd[:, :, 1:3], xg[:, :, 2:4])
                nc.vector.tensor_mul(pab[:, :, 3:4], xg[:, :, 0:1], xg[:, :, 1:2])
                nc.vector.tensor_mul(pcd[:, :, 3:4], xg[:, :, 2:3], xg[:, :, 3:4])
                # fold S_c into group 0 of each chunk
                for c in range(CHUNK):
                    nc.vector.tensor_scalar(
                        pcd[:, c * NG, 0:4],
                        pcd[:, c * NG, 0:4],
                        s_t[:, c : c + 1],
                        None,
                        OP.mult,
                    )

                # --- Q[b, cg, i, j] = pab x pcd (one op, 512 cols) ---
                q = wpool.tile([TILE_B, CG, 4, 4], F32, tag="q")
                pab_b = pab[:].unsqueeze(3).broadcast_to([TILE_B, CG, 4, 4])
                pcd_b = pcd[:].unsqueeze(2).broadcast_to([TILE_B, CG, 4, 4])
                nc.vector.tensor_tensor(q[:], pab_b, pcd_b, OP.mult)

                # --- transpose Q (one [128,128] per chunk) -> wide PSUM ---
                qf = q[:].rearrange("p cg i j -> p (cg i j)")  # [128, 2048]
                ps_a = tps.tile([128, MACRO_B], F32, tag="ps_a")
                for c in range(CHUNK):
                    cw = slice(c * TILE_B, (c + 1) * TILE_B)
                    nc.tensor.transpose(
                        ps_a[:, cw], qf[:, c * 128 : (c + 1) * 128], id_sb[:]
                    )

                # --- copy QT halves to SBUF (2 wide ScalarE copies) ---
                # qts[t] rows: groups 4t..4t+3, 16 monomial rows each
                qts = [
                    qpool.tile([64, MACRO_B], F32, tag=f"qt{h}", name=f"qt{h}")
                    for h in range(2)
                ]
                nc.scalar.copy(qts[0][:], ps_a[0:64, :])
                nc.scalar.copy(qts[1][:], ps_a[64:128, :])

                # --- 8 group matmuls (K=32) + product chain ---
                # even groups: PSUM->SBUF copy on ScalarE; odd groups:
                # DVE multiplies PSUM x SBUF; GPSIMD folds the SBUF tree.
                g_ps = [
                    gps.tile([RU, MACRO_B], F32, tag=f"g{i}", name=f"g{i}")
                    for i in range(2)
                ]
                a_sb = [
                    qpool.tile([RU, MACRO_B], F32, tag=f"a{i}", name=f"a{i}")
                    for i in range(4)
                ]
                t_sb = [
                    qpool.tile([RU, MACRO_B], F32, tag=f"t{i}", name=f"t{i}")
                    for i in range(4)
                ]
                u_sb = [
                    qpool.tile([RU, MACRO_B], F32, tag=f"u{i}", name=f"u{i}")
                    for i in range(2)
                ]
                prod = qpool.tile([RU, MACRO_B], F32, tag="prod")
                for g in range(NG):
                    h, k = g // 2, g % 2
                    qt = qts[g // 4]
                    go = 32 * ((g % 4) // 2)  # == 32*(h%2)
                    csb = c_sb[h // 2]
                    dst = g_ps[g % 2]
                    nc.tensor.matmul(
                        dst[:],
                        csb[go : go + 32, RU * k : RU * (k + 1)],
                        qt[go : go + 32, :],
                        start=True,
                        stop=True,
                    )
                    # even groups: evacuate PSUM on ScalarE; odd: DVE mult
                    if g % 2 == 0:
                        nc.scalar.copy(a_sb[g // 2][:], dst[:])
                    else:
                        nc.vector.tensor_mul(
                            t_sb[g // 2][:], a_sb[g // 2][:], dst[:]
                        )
                nc.vector.tensor_mul(u_sb[0][:], t_sb[0][:], t_sb[1][:])
                nc.gpsimd.tensor_mul(u_sb[1][:], t_sb[2][:], t_sb[3][:])
                nc.vector.tensor_mul(prod[:], u_sb[0][:], u_sb[1][:])

                # --- sum over rank: out[u, b] = rind.T @ prod ---
                o_ps = ops_.tile([U, MACRO_B], F32, tag="o_ps")
                nc.tensor.matmul(o_ps[:], ri_sb[:], prod[:], start=True, stop=True)
                o_sb = qpool.tile([U, MACRO_B], F32, tag="o_sb")
                nc.scalar.copy(o_sb[:], o_ps[:])
                nc.sync.dma_start(out=out[mi], in_=o_sb[:])
    nc.finalize()
    return nc


def _pack_weights(kernel: np.ndarray):
    K = kernel.astype(np.float32)  # [2, R, F, U]
    C = np.zeros((128, 2 * RU), np.float32)
    bits = [(0, 0), (1, 0), (0, 1), (1, 1)]
    for g in range(NG):
        h, k = g // 2, g % 2
        r0 = 64 * (h // 2) + 32 * (h % 2) + 16 * k
        c0 = RU * k
        fs = [4 * g, 4 * g + 1, 4 * g + 2, 4 * g + 3]
        for i, (ba, bb) in enumerate(bits):
            for j, (bc, bd) in enumerate(bits):
                coef = (
                    K[ba, :, fs[0], :]
                    * K[bb, :, fs[1], :]
                    * K[bc, :, fs[2], :]
                    * K[bd, :, fs[3], :]
                )  # [R, U]
                C[r0 + i * 4 + j, c0 : c0 + RU] = coef.reshape(RU)
    ident = np.eye(128, dtype=np.float32)
    rind = np.zeros((RU, U), np.float32)
    for r in range(R):
        for u in range(U):
            rind[r * U + u, u] = 1.0
    return C, ident, rind


_NC_CACHE = {}


def kernel(X: np.ndarray, kernel: np.ndarray) -> np.ndarray:
    if "nc" not in _NC_CACHE:
        _NC_CACHE["nc"] = build_nc()
    nc = _NC_CACHE["nc"]
    C, ident, rind = _pack_weights(kernel)
    X = np.ascontiguousarray(X, dtype=np.float32)
    # [core, macro, chunk, partition, F] -> [core, macro, partition, chunk, F]
    Xd = (
        X.reshape(N_CORES, N_MACRO, CHUNK, TILE_B, F)
        .transpose(0, 1, 3, 2, 4)
        .copy()
    )
    in_maps = []
    for c in range(N_CORES):
        in_maps.append(
            {
                "X": Xd[c],
                "C": C,
                "ident": ident,
                "rind": rind,
            }
        )
    res = run_bass_kernel_spmd(nc, in_maps, core_ids=list(range(N_CORES)))
    outs = []
    for c in range(N_CORES):
        o = res.results[c]["out"]  # [N_MACRO, U, MACRO_B]
        outs.append(o.transpose(0, 2, 1).reshape(B_CORE, U))
    return np.concatenate(outs, axis=0).astype(np.float32)


if __name__ == "__main__":
    rng = np.random.default_rng(0)
    X = rng.standard_normal((B_FULL, F), dtype=np.float32)
    K = (rng.standard_normal((2, R, F, U)) * 0.24).astype(np.float32)
    y = kernel(X, K)
    print(y.shape, y.dtype, np.abs(y).max())



# revision 59
# speedup vs baseline: 2.2736x; 2.2736x over previous
"""Trainium2 Bass kernel for nn_CP_Based (CP-decomposition feature-product layer).

Math: out[b,u] = sum_r prod_f ( x0[b,f]*K[0,r,f,u] + x1[b,f]*K[1,r,f,u] )
  with x0 = 1/sqrt(1+X^2), x1 = X/sqrt(1+X^2).
Factor the normalization out of the f-product:
  out[b,u] = S[b] * sum_r prod_f ( K0[f,ru] + X[b,f]*K1[f,ru] ),
  S[b] = 1/sqrt(prod_f (1+X[b,f]^2)).
The 32-feature product is decomposed into 8 groups of 4 features; each group's
product is linear in the 16 multilinear monomials of its 4 features:
  G_g[b,ru] = sum_m Q_g[b,m] * C_g[m,ru]
with C_g packed on the host (ru columns u-major so the rank-sum is an
innermost-axis reduction).

Layout: batch lives on PSUM partitions. Monomials Q are built batch-major on
DVE in fp16, transposed to monomial-major with a single fused DMA xbar
transpose per macro (one 128x128 block per subtile), and used as the 64-row
stationary operand of fp16 matmuls (four groups share a 64-row block; four
zero-padded C column variants select one group each). Each matmul streams only
the 80 C columns, so PE time is ~35ns per matmul. Group products are strided
elementwise muls over PSUM split across DVE and GPSIMD, the 3-level tree runs
in bf16 on DVE, the rank-sum is a strided tensor_reduce, and S folds in as a
broadcast multiply at the end.

The emission is software-pipelined one macro deep: macro mi's input DMA, S
chain, monomial build, and transpose are emitted before macro mi-1's matmuls,
pair products, tree, and output, so each engine's in-order stream never parks
on the transpose latency.

Sharding: pure data-parallel over batch: 131072 rows -> 8 cores x 16384.
"""

import sys

import ml_dtypes
import numpy as np

sys.path.insert(0, "/opt/trn_rl_repo")

import concourse.bacc as bacc  # noqa: E402
import concourse.mybir as mybir  # noqa: E402
from concourse.bass_utils import run_bass_kernel_spmd  # noqa: E402
from concourse.tile import TileContext  # noqa: E402

F32 = mybir.dt.float32
F16 = mybir.dt.float16
BF16 = mybir.dt.bfloat16
AF = mybir.ActivationFunctionType
OP = mybir.AluOpType
AX = mybir.AxisListType

B_FULL = 131072
N_CORES = 8
B_CORE = B_FULL // N_CORES  # 16384
F = 32
R, U = 10, 8
RU = R * U  # 80
NG = 8  # feature groups of 4
TILE_B = 128
CHUNK = 8  # b-subtiles per macro tile
MACRO_B = TILE_B * CHUNK  # 1024
N_MACRO = B_CORE // MACRO_B  # 16
CG = CHUNK * NG  # 64 (chunk, group) pairs

# NOTE: GPSIMD cannot access PSUM on real hardware (BIR verifier), so all
# pair-products (PSUM readers) run on DVE; GPSIMD gets SBUF-only tree work.
RANKSUM_PE = False


def build_nc():
    nc = bacc.Bacc()
    # host pre-arranges X as [macro, partition, chunk, feature]
    X = nc.dram_tensor(
        "X", [N_MACRO, TILE_B, CHUNK, F], F32, kind="ExternalInput"
    )
    # C[128, 640] fp16: rows 64k..64k+63 hold the 16 monomial rows of groups
    # 4k..4k+3; column block vv*160 holds [C_{4k+2vv} | C_{4k+2vv+1}] with all
    # other rows zero, so each K=64 matmul yields one adjacent group PAIR.
    C2 = nc.dram_tensor("C2", [128, 8 * RU], F16, kind="ExternalInput")
    RIND = nc.dram_tensor("RIND", [RU, U], BF16, kind="ExternalInput")
    out = nc.dram_tensor(
        "out", [N_MACRO, TILE_B, CHUNK, U], F32, kind="ExternalOutput"
    )

    with TileContext(nc) as tc:
        with (
            tc.tile_pool(name="const", bufs=1) as cpool,
            tc.tile_pool(name="xin", bufs=6) as xpool,
            tc.tile_pool(name="bld", bufs=5) as bpool,
            tc.tile_pool(name="qts", bufs=4) as qpool,
            tc.tile_pool(name="tree", bufs=3) as tpool,
            tc.tile_pool(name="outp", bufs=4) as opool,
            tc.tile_pool(name="ps_g", bufs=2, space="PSUM") as gps,
        ):
            ops_ = gps  # only used when RANKSUM_PE
            c_sb = cpool.tile([128, 8 * RU], F16, tag="c2")
            nc.sync.dma_start(out=c_sb[:], in_=C2[:, :])
            ri_sb = cpool.tile([RU, U], BF16, tag="ri")
            nc.sync.dma_start(out=ri_sb[:], in_=RIND[:, :])

            def front(mi):
                """DMA in, S chain, monomial build, fused transpose."""
                st = {}
                xm = xpool.tile([TILE_B, CHUNK, F], F32, tag="x")
                nc.sync.dma_start(out=xm[:], in_=X[mi])

                sq = bpool.tile([TILE_B, CHUNK, F], F32, tag="sq")
                sq1 = bpool.tile([TILE_B, CHUNK, F], F32, tag="sq1")
                s_p = bpool.tile([TILE_B, CHUNK], F32, tag="s_p")
                s_r = bpool.tile([TILE_B, CHUNK], F32, tag="s_r")
                s_t = bpool.tile([TILE_B, CHUNK], F32, tag="s_t")
                nc.scalar.activation(sq[:], xm[:], AF.Square)
                nc.vector.tensor_scalar_add(sq1[:], sq[:], 1.0)
                nc.vector.tensor_reduce(s_p[:], sq1[:], AX.X, OP.mult)
                nc.vector.reciprocal(s_r[:], s_p[:])
                nc.scalar.activation(s_t[:], s_r[:], AF.Sqrt)
                st["s_t"] = s_t

                # pabcd[128, cg, 2, 4]: [.,.,0,:] = (1, Xa, Xb, XaXb),
                #                       [.,.,1,:] = (1, Xc, Xd, XcXd)
                pabcd = bpool.tile([TILE_B, CG, 2, 4], F16, tag="pabcd")
                xv = xm[:].rearrange("p c (g t j) -> p (c g) t j", t=2, j=2)
                if mi < 5:  # ones-cols persist across the 5-buf ring
                    nc.gpsimd.memset(pabcd[:, :, :, 0:1], 1.0)
                nc.vector.tensor_copy(pabcd[:, :, :, 1:3], xv)
                nc.vector.tensor_mul(
                    pabcd[:, :, :, 3:4], xv[:, :, :, 0:1], xv[:, :, :, 1:2]
                )

                # replicate pcd along ab (packed output unlocks DVE 2x for q)
                pcdr = bpool.tile([TILE_B, CG, 4, 4], F16, tag="pcdr")
                nc.vector.tensor_copy(
                    pcdr[:],
                    pabcd[:, :, 1, :].unsqueeze(3).broadcast_to([TILE_B, CG, 4, 4]),
                )
                # q[b, cg, cd, ab] = pcdr x pab (one 2x op, 1024 cols)
                q = bpool.tile([TILE_B, CHUNK, NG, 4, 4], F16, tag="q")
                qcg = q[:].rearrange("p c g i j -> p (c g) i j")
                pab_b = (
                    pabcd[:, :, 0, :].unsqueeze(2).broadcast_to([TILE_B, CG, 4, 4])
                )
                nc.vector.tensor_tensor(qcg, pcdr[:], pab_b, OP.mult)

                # one fused xbar transpose: qt[:, c, :] = q-block-c ^T
                qt = qpool.tile([128, CHUNK, TILE_B], F16, tag="qt")
                nc.sync.dma_start_transpose(
                    qt[:], q[:].rearrange("p c g i j -> p (c g i j)")
                )
                st["qt"] = qt
                return st

            H = CHUNK // 2

            def pairs_stage(mi, st):
                """Matmul waves; Act evacuations; Pool pairs (doubles 0,1).

                DVE pairs for doubles 2,3 are emitted later (dve_pairs) so
                the DVE stream does not park here. HW rules honored: GPSIMD
                never touches PSUM; DVE reads at most one PSUM operand.
                """
                qt = st["qt"]
                # prod is RU-padded to 128 cols so the whole tile
                # xbar-transposes; cols RU:128 are never written (the PE
                # rank-sum contracts K=RU only)
                st["prod"] = tpool.tile(
                    [TILE_B, CHUNK, 128], BF16, tag="prod", name="prod"
                )
                st["rr"] = opool.tile(
                    [TILE_B, CHUNK, U], F32, tag="rr", name="rr"
                )
                st["t"] = tpool.tile(
                    [TILE_B, CHUNK, 4, RU], BF16, tag="t", name="t"
                )
                st["g"] = {}
                for di in range(CHUNK // 2):
                    # one PSUM tile covers 2 subtiles x 4 group-pair blocks
                    g_ps = gps.tile([TILE_B, 2, 4, 256], F32, tag="G")
                    for si in range(2):
                        c = 2 * di + si
                        for kk in range(2):
                            for vv in range(2):
                                nc.tensor.matmul(
                                    g_ps[:, si, 2 * kk + vv, 0 : 2 * RU],
                                    qt[64 * kk : 64 * (kk + 1), c, :],
                                    c_sb[
                                        64 * kk : 64 * (kk + 1),
                                        2 * RU * vv : 2 * RU * (vv + 1),
                                    ],
                                    start=True,
                                    stop=True,
                                )
                    tt = st["t"][:, 2 * di : 2 * di + 2].rearrange(
                        "p s k f -> p (s k) f"
                    )
                    if di < 2:
                        # full evac on Act, then SBUF-only pair on GPSIMD
                        ef = tpool.tile(
                            [TILE_B, 2, 4, 2, RU], BF16, tag=f"ef{di}",
                            name="ef",
                        )
                        nc.scalar.copy(
                            ef[:],
                            g_ps[:, :, :, 0 : 2 * RU].rearrange(
                                "p s k (i f) -> p s k i f", f=RU
                            ),
                        )
                        nc.gpsimd.tensor_mul(
                            tt,
                            ef[:, :, :, 0, :].rearrange("p s k f -> p (s k) f"),
                            ef[:, :, :, 1, :].rearrange("p s k f -> p (s k) f"),
                        )
                    else:
                        # evens evac on Act; mixed SBUF x PSUM pair on DVE
                        ee = tpool.tile(
                            [TILE_B, 2, 4, RU], BF16, tag=f"ee{di}", name="ee"
                        )
                        nc.scalar.copy(ee[:], g_ps[:, :, :, 0:RU])
                        st["g"][di] = (g_ps, ee)

            def dve_pairs(mi, st):
                for di, (g_ps, ee) in st["g"].items():
                    tt = st["t"][:, 2 * di : 2 * di + 2].rearrange(
                        "p s k f -> p (s k) f"
                    )
                    nc.vector.tensor_mul(
                        tt,
                        ee[:].rearrange("p s k f -> p (s k) f"),
                        g_ps[:, :, :, RU : 2 * RU].rearrange(
                            "p s k f -> p (s k) f"
                        ),
                    )

            def tree_stage(mi, st, h):
                cs = slice(h * H, (h + 1) * H)
                u_sb = tpool.tile([TILE_B, H, 2, RU], BF16, tag=f"u{h}")
                tv = st["t"][:, cs].rearrange(
                    "p c (i par) f -> p c i par f", par=2
                )
                if h == 0:  # SBUF-only tree half on GPSIMD
                    nc.gpsimd.tensor_mul(
                        u_sb[:], tv[:, :, :, 0, :], tv[:, :, :, 1, :]
                    )
                    nc.gpsimd.tensor_mul(
                        st["prod"][:, cs, 0:RU],
                        u_sb[:, :, 0, :],
                        u_sb[:, :, 1, :],
                    )
                else:
                    nc.vector.tensor_mul(
                        u_sb[:], tv[:, :, :, 0, :], tv[:, :, :, 1, :]
                    )
                    nc.vector.tensor_mul(
                        st["prod"][:, cs, 0:RU],
                        u_sb[:, :, 0, :],
                        u_sb[:, :, 1, :],
                    )
                if not RANKSUM_PE:
                    nc.vector.tensor_reduce(
                        st["rr"][:, cs],
                        st["prod"][:, cs, 0:RU].rearrange(
                            "p c (u r) -> p c u r", r=R
                        ),
                        AX.X,
                        OP.add,
                    )

            def xpose_prod(mi, st):
                if RANKSUM_PE:
                    # transpose prod -> [ru, c, b] (rank-sum runs next stage)
                    pt = qpool.tile([128, CHUNK, TILE_B], BF16, tag="pt")
                    nc.sync.dma_start_transpose(
                        pt[:], st["prod"][:].rearrange("p c f -> p (c f)")
                    )
                    st["pt"] = pt

            def last(mi, st):
                s_t = st["s_t"]
                o_sb = opool.tile([TILE_B, CHUNK, U], F32, tag="o")
                if RANKSUM_PE:
                    pt = st["pt"]
                    o_ps = ops_.tile([TILE_B, CHUNK, U], F32, tag="o_ps")
                    for c in range(CHUNK):
                        nc.tensor.matmul(
                            o_ps[:, c, :],
                            pt[0:RU, c, :],
                            ri_sb[:],
                            start=True,
                            stop=True,
                        )
                    rr = o_ps
                else:
                    rr = st["rr"]
                nc.gpsimd.tensor_tensor(
                    o_sb[:],
                    rr[:],
                    s_t[:].unsqueeze(2).broadcast_to([TILE_B, CHUNK, U]),
                    OP.mult,
                )
                nc.sync.dma_start(out=out[mi], in_=o_sb[:])

            DEPTH = 3  # front -> matmul/pair stage lookahead (macros)
            pend = {}
            for mi in range(N_MACRO + DEPTH + 2):
                if mi >= DEPTH + 2:
                    last(mi - DEPTH - 2, pend.pop(mi - DEPTH - 2))
                if mi >= DEPTH and mi - DEPTH < N_MACRO:
                    pairs_stage(mi - DEPTH, pend[mi - DEPTH])
                if mi >= DEPTH + 1 and mi - DEPTH - 1 < N_MACRO:
                    xpose_prod(mi - DEPTH - 1, pend[mi - DEPTH - 1])
                if mi < N_MACRO:
                    pend[mi] = front(mi)
                if mi >= DEPTH and mi - DEPTH < N_MACRO:
                    dve_pairs(mi - DEPTH, pend[mi - DEPTH])
                    tree_stage(mi - DEPTH, pend[mi - DEPTH], 0)
                    tree_stage(mi - DEPTH, pend[mi - DEPTH], 1)
    nc.finalize()
    return nc


def _pack_weights(kernel: np.ndarray):
    K = kernel.astype(np.float64)  # [2, R, F, U]
    C2 = np.zeros((128, 8 * RU), np.float32)
    for g in range(NG):
        kk, vv, side = g // 4, (g % 4) // 2, g % 2
        r0 = 64 * kk + 16 * (g % 4)
        c0 = 2 * RU * vv + RU * side
        fs = [4 * g, 4 * g + 1, 4 * g + 2, 4 * g + 3]
        for m in range(16):
            ab, cd = m % 4, m // 4
            bits = (ab & 1, (ab >> 1) & 1, cd & 1, (cd >> 1) & 1)
            coef = (
                K[bits[0], :, fs[0], :]
                * K[bits[1], :, fs[1], :]
                * K[bits[2], :, fs[2], :]
                * K[bits[3], :, fs[3], :]
            )  # [R, U]
            # ru columns u-major: col = u*R + r
            C2[r0 + m, c0 : c0 + RU] = coef.T.reshape(RU)
    rind = np.zeros((RU, U), np.float32)
    for u in range(U):
        rind[u * R : (u + 1) * R, u] = 1.0
    return C2.astype(np.float16), rind


_NC_CACHE = {}


def kernel(X: np.ndarray, kernel: np.ndarray) -> np.ndarray:
    if "nc" not in _NC_CACHE:
        _NC_CACHE["nc"] = build_nc()
    nc = _NC_CACHE["nc"]
    C2, rind = _pack_weights(kernel)
    X = np.ascontiguousarray(X, dtype=np.float32)
    # [core, macro, chunk, partition, F] -> [core, macro, partition, chunk, F]
    Xd = (
        X.reshape(N_CORES, N_MACRO, CHUNK, TILE_B, F)
        .transpose(0, 1, 3, 2, 4)
        .copy()
    )
    in_maps = []
    for c in range(N_CORES):
        in_maps.append(
            {"X": Xd[c], "C2": C2, "RIND": rind.astype(ml_dtypes.bfloat16)}
        )
    res = run_bass_kernel_spmd(nc, in_maps, core_ids=list(range(N_CORES)))
    outs = []
    for c in range(N_CORES):
        o = res.results[c]["out"]  # [N_MACRO, TILE_B, CHUNK, U]
        outs.append(o.transpose(0, 2, 1, 3).reshape(B_CORE, U))
    return np.concatenate(outs, axis=0).astype(np.float32)


if __name__ == "__main__":
    rng = np.random.default_rng(0)
    X = rng.standard_normal((B_FULL, F), dtype=np.float32)
    K = (rng.standard_normal((2, R, F, U)) * 0.24).astype(np.float32)
    y = kernel(X, K)
    print(y.shape, y.dtype, np.abs(y).max())


# revision 62
# speedup vs baseline: 2.5972x; 1.1423x over previous
"""Trainium2 Bass kernel for nn_CP_Based (CP-decomposition feature-product layer).

Math: out[b,u] = sum_r prod_f ( x0[b,f]*K[0,r,f,u] + x1[b,f]*K[1,r,f,u] )
  with x0 = 1/sqrt(1+X^2), x1 = X/sqrt(1+X^2).
Factor the normalization out of the f-product:
  out[b,u] = S[b] * sum_r prod_f ( K0[f,ru] + X[b,f]*K1[f,ru] ),
  S[b] = 1/sqrt(prod_f (1+X[b,f]^2)).
The 32-feature product is decomposed into 8 groups of 4 features; each group's
product is linear in the 16 multilinear monomials of its 4 features:
  G_g[b,ru] = sum_m Q_g[b,m] * C_g[m,ru]
with C_g packed on the host (ru columns u-major so the rank-sum is an
innermost-axis reduction).

Layout: batch lives on PSUM partitions. Monomials Q are built batch-major on
DVE in fp16, transposed to monomial-major with a single fused DMA xbar
transpose per macro (one 128x128 block per subtile), and used as the 64-row
stationary operand of fp16 matmuls. Four groups share a 64-row block and the
zero-padded C column blocks select one adjacent group PAIR per matmul, so each
K=64 matmul streams just 160 C columns (~70ns of PE). Real-HW constraints
shape the pair-product phase: GPSIMD cannot touch PSUM and DVE may read only
one PSUM operand per op, so two subtile-doubles get a full Act (ScalarE)
evacuation to bf16 with the pair-mul on GPSIMD/DVE, and two get an Act
evens-evacuation with a mixed SBUF x PSUM pair-mul on DVE. The bf16 tree
(4->2->1) is split across GPSIMD and DVE, the rank-sum is a strided
tensor_reduce on DVE (ru columns are packed u-major so rank is innermost),
and S folds in as a broadcast multiply on GPSIMD.

The emission is software-pipelined DEPTH=3 macros deep with the per-macro
phases (front build / matmul+evac+pairs / DVE pairs / tree / rank-sum+store)
interleaved so no engine's in-order stream parks on a cross-engine latency.

Sharding: pure data-parallel over batch: 131072 rows -> 8 cores x 16384.
"""

import sys

import ml_dtypes
import numpy as np

sys.path.insert(0, "/opt/trn_rl_repo")

import concourse.bacc as bacc  # noqa: E402
import concourse.mybir as mybir  # noqa: E402
from concourse.bass_utils import run_bass_kernel_spmd  # noqa: E402
from concourse.tile import TileContext  # noqa: E402

F32 = mybir.dt.float32
F16 = mybir.dt.float16
BF16 = mybir.dt.bfloat16
AF = mybir.ActivationFunctionType
OP = mybir.AluOpType
AX = mybir.AxisListType

B_FULL = 131072
N_CORES = 8
B_CORE = B_FULL // N_CORES  # 16384
F = 32
R, U = 10, 8
RU = R * U  # 80
NG = 8  # feature groups of 4
TILE_B = 128
CHUNK = 8  # b-subtiles per macro tile
MACRO_B = TILE_B * CHUNK  # 1024
N_MACRO = B_CORE // MACRO_B  # 16
CG = CHUNK * NG  # 64 (chunk, group) pairs

# NOTE: GPSIMD cannot access PSUM on real hardware (BIR verifier), so all
# pair-products (PSUM readers) run on DVE; GPSIMD gets SBUF-only tree work.
RANKSUM_PE = False


def build_nc():
    nc = bacc.Bacc()
    # host pre-arranges X as [macro, partition, chunk, feature]
    X = nc.dram_tensor(
        "X", [N_MACRO, TILE_B, CHUNK, F], F32, kind="ExternalInput"
    )
    # C[128, 640] fp16: rows 64k..64k+63 hold the 16 monomial rows of groups
    # 4k..4k+3; column block vv*160 holds [C_{4k+2vv} | C_{4k+2vv+1}] with all
    # other rows zero, so each K=64 matmul yields one adjacent group PAIR.
    C2 = nc.dram_tensor("C2", [128, 8 * RU], F16, kind="ExternalInput")
    RIND = nc.dram_tensor("RIND", [RU, U], BF16, kind="ExternalInput")
    out = nc.dram_tensor(
        "out", [N_MACRO, TILE_B, CHUNK, U], F32, kind="ExternalOutput"
    )

    with TileContext(nc) as tc:
        with (
            tc.tile_pool(name="const", bufs=1) as cpool,
            tc.tile_pool(name="xin", bufs=6) as xpool,
            tc.tile_pool(name="bld", bufs=5) as bpool,
            tc.tile_pool(name="qts", bufs=4) as qpool,
            tc.tile_pool(name="tree", bufs=3) as tpool,
            tc.tile_pool(name="outp", bufs=4) as opool,
            tc.tile_pool(name="ps_g", bufs=2, space="PSUM") as gps,
        ):
            ops_ = gps  # only used when RANKSUM_PE
            c_sb = cpool.tile([128, 8 * RU], F16, tag="c2")
            nc.sync.dma_start(out=c_sb[:], in_=C2[:, :])
            ri_sb = cpool.tile([RU, U], BF16, tag="ri")
            nc.sync.dma_start(out=ri_sb[:], in_=RIND[:, :])

            def front(mi):
                """DMA in, S chain, monomial build, fused transpose."""
                st = {}
                xm = xpool.tile([TILE_B, CHUNK, F], F32, tag="x")
                nc.sync.dma_start(out=xm[:], in_=X[mi])

                sq = bpool.tile([TILE_B, CHUNK, F], F32, tag="sq")
                sq1 = bpool.tile([TILE_B, CHUNK, F], F32, tag="sq1")
                s_p = bpool.tile([TILE_B, CHUNK], F32, tag="s_p")
                s_r = bpool.tile([TILE_B, CHUNK], F32, tag="s_r")
                s_t = bpool.tile([TILE_B, CHUNK], F32, tag="s_t")
                nc.scalar.activation(sq[:], xm[:], AF.Square)
                nc.vector.tensor_scalar_add(sq1[:], sq[:], 1.0)
                nc.vector.tensor_reduce(s_p[:], sq1[:], AX.X, OP.mult)
                nc.vector.reciprocal(s_r[:], s_p[:])
                nc.scalar.activation(s_t[:], s_r[:], AF.Sqrt)
                st["s_t"] = s_t

                # pabcd[128, cg, 2, 4]: [.,.,0,:] = (1, Xa, Xb, XaXb),
                #                       [.,.,1,:] = (1, Xc, Xd, XcXd)
                pabcd = bpool.tile([TILE_B, CG, 2, 4], F16, tag="pabcd")
                xv = xm[:].rearrange("p c (g t j) -> p (c g) t j", t=2, j=2)
                if mi < 5:  # ones-cols persist across the 5-buf ring
                    nc.gpsimd.memset(pabcd[:, :, :, 0:1], 1.0)
                nc.vector.tensor_copy(pabcd[:, :, :, 1:3], xv)
                nc.vector.tensor_mul(
                    pabcd[:, :, :, 3:4], xv[:, :, :, 0:1], xv[:, :, :, 1:2]
                )

                # replicate pcd along ab (packed output unlocks DVE 2x for q)
                pcdr = bpool.tile([TILE_B, CG, 4, 4], F16, tag="pcdr")
                nc.scalar.copy(
                    pcdr[:],
                    pabcd[:, :, 1, :].unsqueeze(3).broadcast_to([TILE_B, CG, 4, 4]),
                )
                # q[b, cg, cd, ab] = pcdr x pab (one 2x op, 1024 cols)
                q = bpool.tile([TILE_B, CHUNK, NG, 4, 4], F16, tag="q")
                qcg = q[:].rearrange("p c g i j -> p (c g) i j")
                pab_b = (
                    pabcd[:, :, 0, :].unsqueeze(2).broadcast_to([TILE_B, CG, 4, 4])
                )
                nc.vector.tensor_tensor(qcg, pcdr[:], pab_b, OP.mult)

                # one fused xbar transpose: qt[:, c, :] = q-block-c ^T
                qt = qpool.tile([128, CHUNK, TILE_B], F16, tag="qt")
                nc.sync.dma_start_transpose(
                    qt[:], q[:].rearrange("p c g i j -> p (c g i j)")
                )
                st["qt"] = qt
                return st

            H = CHUNK // 2

            def pairs_stage(mi, st):
                """Matmul waves; Act evacuations; Pool pairs (doubles 0,1).

                DVE pairs for doubles 2,3 are emitted later (dve_pairs) so
                the DVE stream does not park here. HW rules honored: GPSIMD
                never touches PSUM; DVE reads at most one PSUM operand.
                """
                qt = st["qt"]
                # prod is RU-padded to 128 cols so the whole tile
                # xbar-transposes; cols RU:128 are never written (the PE
                # rank-sum contracts K=RU only)
                st["prod"] = tpool.tile(
                    [TILE_B, CHUNK, 128], BF16, tag="prod", name="prod"
                )
                st["rr"] = opool.tile(
                    [TILE_B, CHUNK, U], F32, tag="rr", name="rr"
                )
                st["t"] = tpool.tile(
                    [TILE_B, CHUNK, 4, RU], BF16, tag="t", name="t"
                )
                st["g"] = {}
                for di in range(CHUNK // 2):
                    # one PSUM tile covers 2 subtiles x 4 group-pair blocks
                    g_ps = gps.tile([TILE_B, 2, 4, 256], F32, tag="G")
                    for si in range(2):
                        c = 2 * di + si
                        for kk in range(2):
                            for vv in range(2):
                                nc.tensor.matmul(
                                    g_ps[:, si, 2 * kk + vv, 0 : 2 * RU],
                                    qt[64 * kk : 64 * (kk + 1), c, :],
                                    c_sb[
                                        64 * kk : 64 * (kk + 1),
                                        2 * RU * vv : 2 * RU * (vv + 1),
                                    ],
                                    start=True,
                                    stop=True,
                                )
                    tt = st["t"][:, 2 * di : 2 * di + 2].rearrange(
                        "p s k f -> p (s k) f"
                    )
                    if di < 2:
                        # full evac on Act, then SBUF-only pair on GPSIMD
                        ef = tpool.tile(
                            [TILE_B, 2, 4, 2, RU], BF16, tag=f"ef{di}",
                            name="ef",
                        )
                        nc.scalar.copy(
                            ef[:],
                            g_ps[:, :, :, 0 : 2 * RU].rearrange(
                                "p s k (i f) -> p s k i f", f=RU
                            ),
                        )
                        peng = nc.gpsimd if di == 0 else nc.vector
                        peng.tensor_mul(
                            tt,
                            ef[:, :, :, 0, :].rearrange("p s k f -> p (s k) f"),
                            ef[:, :, :, 1, :].rearrange("p s k f -> p (s k) f"),
                        )
                    else:
                        # evens evac on Act; mixed SBUF x PSUM pair on DVE
                        ee = tpool.tile(
                            [TILE_B, 2, 4, RU], BF16, tag=f"ee{di}", name="ee"
                        )
                        nc.scalar.copy(ee[:], g_ps[:, :, :, 0:RU])
                        st["g"][di] = (g_ps, ee)

            def dve_pairs(mi, st):
                for di, (g_ps, ee) in st["g"].items():
                    tt = st["t"][:, 2 * di : 2 * di + 2].rearrange(
                        "p s k f -> p (s k) f"
                    )
                    nc.vector.tensor_mul(
                        tt,
                        ee[:].rearrange("p s k f -> p (s k) f"),
                        g_ps[:, :, :, RU : 2 * RU].rearrange(
                            "p s k f -> p (s k) f"
                        ),
                    )

            def tree_stage(mi, st, h):
                cs = slice(h * H, (h + 1) * H)
                u_sb = tpool.tile([TILE_B, H, 2, RU], BF16, tag=f"u{h}")
                tv = st["t"][:, cs].rearrange(
                    "p c (i par) f -> p c i par f", par=2
                )
                if h == 0:  # SBUF-only tree half on GPSIMD
                    nc.gpsimd.tensor_mul(
                        u_sb[:], tv[:, :, :, 0, :], tv[:, :, :, 1, :]
                    )
                    nc.vector.tensor_mul(
                        st["prod"][:, cs, 0:RU],
                        u_sb[:, :, 0, :],
                        u_sb[:, :, 1, :],
                    )
                else:
                    nc.vector.tensor_mul(
                        u_sb[:], tv[:, :, :, 0, :], tv[:, :, :, 1, :]
                    )
                    nc.vector.tensor_mul(
                        st["prod"][:, cs, 0:RU],
                        u_sb[:, :, 0, :],
                        u_sb[:, :, 1, :],
                    )
                if not RANKSUM_PE:
                    nc.vector.tensor_reduce(
                        st["rr"][:, cs],
                        st["prod"][:, cs, 0:RU].rearrange(
                            "p c (u r) -> p c u r", r=R
                        ),
                        AX.X,
                        OP.add,
                    )

            def xpose_prod(mi, st):
                if RANKSUM_PE:
                    # transpose prod -> [ru, c, b] (rank-sum runs next stage)
                    pt = qpool.tile([128, CHUNK, TILE_B], BF16, tag="pt")
                    nc.sync.dma_start_transpose(
                        pt[:], st["prod"][:].rearrange("p c f -> p (c f)")
                    )
                    st["pt"] = pt

            def last(mi, st):
                s_t = st["s_t"]
                o_sb = opool.tile([TILE_B, CHUNK, U], F32, tag="o")
                if RANKSUM_PE:
                    pt = st["pt"]
                    o_ps = ops_.tile([TILE_B, CHUNK, U], F32, tag="o_ps")
                    for c in range(CHUNK):
                        nc.tensor.matmul(
                            o_ps[:, c, :],
                            pt[0:RU, c, :],
                            ri_sb[:],
                            start=True,
                            stop=True,
                        )
                    rr = o_ps
                else:
                    rr = st["rr"]
                nc.gpsimd.tensor_tensor(
                    o_sb[:],
                    rr[:],
                    s_t[:].unsqueeze(2).broadcast_to([TILE_B, CHUNK, U]),
                    OP.mult,
                )
                nc.sync.dma_start(out=out[mi], in_=o_sb[:])

            DEPTH = 3  # front -> matmul/pair stage lookahead (macros)
            pend = {}
            for mi in range(N_MACRO + DEPTH + 2):
                if mi >= DEPTH + 2:
                    last(mi - DEPTH - 2, pend.pop(mi - DEPTH - 2))
                if mi >= DEPTH and mi - DEPTH < N_MACRO:
                    pairs_stage(mi - DEPTH, pend[mi - DEPTH])
                if mi >= DEPTH + 1 and mi - DEPTH - 1 < N_MACRO:
                    xpose_prod(mi - DEPTH - 1, pend[mi - DEPTH - 1])
                if mi < N_MACRO:
                    pend[mi] = front(mi)
                if mi >= DEPTH and mi - DEPTH < N_MACRO:
                    dve_pairs(mi - DEPTH, pend[mi - DEPTH])
                    tree_stage(mi - DEPTH, pend[mi - DEPTH], 0)
                    tree_stage(mi - DEPTH, pend[mi - DEPTH], 1)
    nc.finalize()
    return nc


def _pack_weights(kernel: np.ndarray):
    K = kernel.astype(np.float64)  # [2, R, F, U]
    C2 = np.zeros((128, 8 * RU), np.float32)
    for g in range(NG):
        kk, vv, side = g // 4, (g % 4) // 2, g % 2
        r0 = 64 * kk + 16 * (g % 4)
        c0 = 2 * RU * vv + RU * side
        fs = [4 * g, 4 * g + 1, 4 * g + 2, 4 * g + 3]
        for m in range(16):
            ab, cd = m % 4, m // 4
            bits = (ab & 1, (ab >> 1) & 1, cd & 1, (cd >> 1) & 1)
            coef = (
                K[bits[0], :, fs[0], :]
                * K[bits[1], :, fs[1], :]
                * K[bits[2], :, fs[2], :]
                * K[bits[3], :, fs[3], :]
            )  # [R, U]
            # ru columns u-major: col = u*R + r
            C2[r0 + m, c0 : c0 + RU] = coef.T.reshape(RU)
    rind = np.zeros((RU, U), np.float32)
    for u in range(U):
        rind[u * R : (u + 1) * R, u] = 1.0
    return C2.astype(np.float16), rind


_NC_CACHE = {}


def kernel(X: np.ndarray, kernel: np.ndarray) -> np.ndarray:
    if "nc" not in _NC_CACHE:
        _NC_CACHE["nc"] = build_nc()
    nc = _NC_CACHE["nc"]
    C2, rind = _pack_weights(kernel)
    X = np.ascontiguousarray(X, dtype=np.float32)
    # [core, macro, chunk, partition, F] -> [core, macro, partition, chunk, F]
    Xd = (
        X.reshape(N_CORES, N_MACRO, CHUNK, TILE_B, F)
        .transpose(0, 1, 3, 2, 4)
        .copy()
    )
    in_maps = []
    for c in range(N_CORES):
        in_maps.append(
            {"X": Xd[c], "C2": C2, "RIND": rind.astype(ml_dtypes.bfloat16)}
        )
    res = run_bass_kernel_spmd(nc, in_maps, core_ids=list(range(N_CORES)))
    outs = []
    for c in range(N_CORES):
        o = res.results[c]["out"]  # [N_MACRO, TILE_B, CHUNK, U]
        outs.append(o.transpose(0, 2, 1, 3).reshape(B_CORE, U))
    return np.concatenate(outs, axis=0).astype(np.float32)


if __name__ == "__main__":
    rng = np.random.default_rng(0)
    X = rng.standard_normal((B_FULL, F), dtype=np.float32)
    K = (rng.standard_normal((2, R, F, U)) * 0.24).astype(np.float32)
    y = kernel(X, K)
    print(y.shape, y.dtype, np.abs(y).max())


# revision 63
# speedup vs baseline: 2.6729x; 1.0291x over previous
"""Trainium2 Bass kernel for nn_CP_Based (CP-decomposition feature-product layer).

Math: out[b,u] = sum_r prod_f ( x0[b,f]*K[0,r,f,u] + x1[b,f]*K[1,r,f,u] )
  with x0 = 1/sqrt(1+X^2), x1 = X/sqrt(1+X^2).
Factor the normalization out of the f-product:
  out[b,u] = S[b] * sum_r prod_f ( K0[f,ru] + X[b,f]*K1[f,ru] ),
  S[b] = 1/sqrt(prod_f (1+X[b,f]^2)).
The 32-feature product is decomposed into 8 groups of 4 features; each group's
product is linear in the 16 multilinear monomials of its 4 features:
  G_g[b,ru] = sum_m Q_g[b,m] * C_g[m,ru]
with C_g packed on the host (ru columns u-major so the rank-sum is an
innermost-axis reduction).

Layout: batch lives on PSUM partitions. Monomials Q are built batch-major on
DVE in fp16, transposed to monomial-major with a single fused DMA xbar
transpose per macro (one 128x128 block per subtile), and used as the 64-row
stationary operand of fp16 matmuls. Four groups share a 64-row block and the
zero-padded C column blocks select one adjacent group PAIR per matmul, so each
K=64 matmul streams just 160 C columns (~70ns of PE). Real-HW constraints
shape the pair-product phase: GPSIMD cannot touch PSUM and DVE may read only
one PSUM operand per op, so two subtile-doubles get a full Act (ScalarE)
evacuation to bf16 with the pair-mul on GPSIMD/DVE, and two get an Act
evens-evacuation with a mixed SBUF x PSUM pair-mul on DVE. The bf16 tree
(4->2->1) is split across GPSIMD and DVE, the rank-sum is a strided
tensor_reduce on DVE (ru columns are packed u-major so rank is innermost),
and S folds in as a broadcast multiply on GPSIMD.

The emission is software-pipelined DEPTH=3 macros deep with the per-macro
phases (front build / matmul+evac+pairs / DVE pairs / tree / rank-sum+store)
interleaved so no engine's in-order stream parks on a cross-engine latency.

Sharding: pure data-parallel over batch: 131072 rows -> 8 cores x 16384.
"""

import sys

import ml_dtypes
import numpy as np

sys.path.insert(0, "/opt/trn_rl_repo")

import concourse.bacc as bacc  # noqa: E402
import concourse.mybir as mybir  # noqa: E402
from concourse.bass_utils import run_bass_kernel_spmd  # noqa: E402
from concourse.tile import TileContext  # noqa: E402

F32 = mybir.dt.float32
F16 = mybir.dt.float16
BF16 = mybir.dt.bfloat16
AF = mybir.ActivationFunctionType
OP = mybir.AluOpType
AX = mybir.AxisListType

B_FULL = 131072
N_CORES = 8
B_CORE = B_FULL // N_CORES  # 16384
F = 32
R, U = 10, 8
RU = R * U  # 80
NG = 8  # feature groups of 4
TILE_B = 128
CHUNK = 8  # b-subtiles per macro tile
MACRO_B = TILE_B * CHUNK  # 1024
N_MACRO = B_CORE // MACRO_B  # 16
CG = CHUNK * NG  # 64 (chunk, group) pairs

# NOTE: GPSIMD cannot access PSUM on real hardware (BIR verifier), so all
# pair-products (PSUM readers) run on DVE; GPSIMD gets SBUF-only tree work.
RANKSUM_PE = False


def build_nc():
    nc = bacc.Bacc()
    # host pre-arranges X as [macro, partition, chunk, feature]
    X = nc.dram_tensor(
        "X", [N_MACRO, TILE_B, CHUNK, F], F32, kind="ExternalInput"
    )
    # C[128, 640] fp16: rows 64k..64k+63 hold the 16 monomial rows of groups
    # 4k..4k+3; column block vv*160 holds [C_{4k+2vv} | C_{4k+2vv+1}] with all
    # other rows zero, so each K=64 matmul yields one adjacent group PAIR.
    C2 = nc.dram_tensor("C2", [128, 8 * RU], F16, kind="ExternalInput")
    RIND = nc.dram_tensor("RIND", [RU, U], BF16, kind="ExternalInput")
    out = nc.dram_tensor(
        "out", [N_MACRO, TILE_B, CHUNK, U], F32, kind="ExternalOutput"
    )

    with TileContext(nc) as tc:
        with (
            tc.tile_pool(name="const", bufs=1) as cpool,
            tc.tile_pool(name="xin", bufs=6) as xpool,
            tc.tile_pool(name="bld", bufs=5) as bpool,
            tc.tile_pool(name="qts", bufs=4) as qpool,
            tc.tile_pool(name="tree", bufs=3) as tpool,
            tc.tile_pool(name="outp", bufs=4) as opool,
            tc.tile_pool(name="ps_g", bufs=2, space="PSUM") as gps,
        ):
            ops_ = gps  # only used when RANKSUM_PE
            c_sb = cpool.tile([128, 8 * RU], F16, tag="c2")
            nc.sync.dma_start(out=c_sb[:], in_=C2[:, :])
            ri_sb = cpool.tile([RU, U], BF16, tag="ri")
            nc.sync.dma_start(out=ri_sb[:], in_=RIND[:, :])

            def front(mi):
                """DMA in, S chain, monomial build, fused transpose."""
                st = {}
                xm = xpool.tile([TILE_B, CHUNK, F], F32, tag="x")
                nc.sync.dma_start(out=xm[:], in_=X[mi])

                sq = bpool.tile([TILE_B, CHUNK, F], F32, tag="sq")
                sq1 = bpool.tile([TILE_B, CHUNK, F], F32, tag="sq1")
                s_p = bpool.tile([TILE_B, CHUNK], F32, tag="s_p")
                s_r = bpool.tile([TILE_B, CHUNK], F32, tag="s_r")
                s_t = bpool.tile([TILE_B, CHUNK], F32, tag="s_t")
                nc.vector.tensor_mul(sq[:], xm[:], xm[:])
                nc.vector.tensor_scalar_add(sq1[:], sq[:], 1.0)
                nc.vector.tensor_reduce(s_p[:], sq1[:], AX.X, OP.mult)
                nc.vector.reciprocal(s_r[:], s_p[:])
                nc.scalar.activation(s_t[:], s_r[:], AF.Sqrt)
                st["s_t"] = s_t

                # pabcd[128, cg, 2, 4]: [.,.,0,:] = (1, Xa, Xb, XaXb),
                #                       [.,.,1,:] = (1, Xc, Xd, XcXd)
                pabcd = bpool.tile([TILE_B, CG, 2, 4], F16, tag="pabcd")
                xv = xm[:].rearrange("p c (g t j) -> p (c g) t j", t=2, j=2)
                if mi < 5:  # ones-cols persist across the 5-buf ring
                    nc.gpsimd.memset(pabcd[:, :, :, 0:1], 1.0)
                nc.vector.tensor_copy(pabcd[:, :, :, 1:3], xv)
                nc.gpsimd.tensor_mul(
                    pabcd[:, :, :, 3:4], xv[:, :, :, 0:1], xv[:, :, :, 1:2]
                )

                # replicate pcd along ab (packed output unlocks DVE 2x for q)
                pcdr = bpool.tile([TILE_B, CG, 4, 4], F16, tag="pcdr")
                nc.scalar.copy(
                    pcdr[:],
                    pabcd[:, :, 1, :].unsqueeze(3).broadcast_to([TILE_B, CG, 4, 4]),
                )
                # q[b, cg, cd, ab] = pcdr x pab (one 2x op, 1024 cols)
                q = bpool.tile([TILE_B, CHUNK, NG, 4, 4], F16, tag="q")
                qcg = q[:].rearrange("p c g i j -> p (c g) i j")
                pab_b = (
                    pabcd[:, :, 0, :].unsqueeze(2).broadcast_to([TILE_B, CG, 4, 4])
                )
                nc.vector.tensor_tensor(qcg, pcdr[:], pab_b, OP.mult)

                # one fused xbar transpose: qt[:, c, :] = q-block-c ^T
                qt = qpool.tile([128, CHUNK, TILE_B], F16, tag="qt")
                nc.sync.dma_start_transpose(
                    qt[:], q[:].rearrange("p c g i j -> p (c g i j)")
                )
                st["qt"] = qt
                return st

            H = CHUNK // 2

            def pairs_stage(mi, st):
                """Matmul waves; Act evacuations; Pool pairs (doubles 0,1).

                DVE pairs for doubles 2,3 are emitted later (dve_pairs) so
                the DVE stream does not park here. HW rules honored: GPSIMD
                never touches PSUM; DVE reads at most one PSUM operand.
                """
                qt = st["qt"]
                # prod is RU-padded to 128 cols so the whole tile
                # xbar-transposes; cols RU:128 are never written (the PE
                # rank-sum contracts K=RU only)
                st["prod"] = tpool.tile(
                    [TILE_B, CHUNK, 128], BF16, tag="prod", name="prod"
                )
                st["rr"] = opool.tile(
                    [TILE_B, CHUNK, U], F32, tag="rr", name="rr"
                )
                st["t"] = tpool.tile(
                    [TILE_B, CHUNK, 4, RU], BF16, tag="t", name="t"
                )
                st["g"] = {}
                for di in range(CHUNK // 2):
                    # one PSUM tile covers 2 subtiles x 4 group-pair blocks
                    g_ps = gps.tile([TILE_B, 2, 4, 256], F32, tag="G")
                    for si in range(2):
                        c = 2 * di + si
                        for kk in range(2):
                            for vv in range(2):
                                nc.tensor.matmul(
                                    g_ps[:, si, 2 * kk + vv, 0 : 2 * RU],
                                    qt[64 * kk : 64 * (kk + 1), c, :],
                                    c_sb[
                                        64 * kk : 64 * (kk + 1),
                                        2 * RU * vv : 2 * RU * (vv + 1),
                                    ],
                                    start=True,
                                    stop=True,
                                )
                    tt = st["t"][:, 2 * di : 2 * di + 2].rearrange(
                        "p s k f -> p (s k) f"
                    )
                    if di < 2:
                        # full evac on Act, then SBUF-only pair on GPSIMD
                        ef = tpool.tile(
                            [TILE_B, 2, 4, 2, RU], BF16, tag=f"ef{di}",
                            name="ef",
                        )
                        nc.scalar.copy(
                            ef[:],
                            g_ps[:, :, :, 0 : 2 * RU].rearrange(
                                "p s k (i f) -> p s k i f", f=RU
                            ),
                        )
                        peng = nc.gpsimd if di == 0 else nc.vector
                        peng.tensor_mul(
                            tt,
                            ef[:, :, :, 0, :].rearrange("p s k f -> p (s k) f"),
                            ef[:, :, :, 1, :].rearrange("p s k f -> p (s k) f"),
                        )
                    else:
                        # evens evac on Act; mixed SBUF x PSUM pair on DVE
                        ee = tpool.tile(
                            [TILE_B, 2, 4, RU], BF16, tag=f"ee{di}", name="ee"
                        )
                        nc.scalar.copy(ee[:], g_ps[:, :, :, 0:RU])
                        st["g"][di] = (g_ps, ee)

            def dve_pairs(mi, st):
                for di, (g_ps, ee) in st["g"].items():
                    tt = st["t"][:, 2 * di : 2 * di + 2].rearrange(
                        "p s k f -> p (s k) f"
                    )
                    nc.vector.tensor_mul(
                        tt,
                        ee[:].rearrange("p s k f -> p (s k) f"),
                        g_ps[:, :, :, RU : 2 * RU].rearrange(
                            "p s k f -> p (s k) f"
                        ),
                    )

            def tree_stage(mi, st, h):
                cs = slice(h * H, (h + 1) * H)
                u_sb = tpool.tile([TILE_B, H, 2, RU], BF16, tag=f"u{h}")
                tv = st["t"][:, cs].rearrange(
                    "p c (i par) f -> p c i par f", par=2
                )
                if h == 0:  # SBUF-only tree half on GPSIMD
                    nc.gpsimd.tensor_mul(
                        u_sb[:], tv[:, :, :, 0, :], tv[:, :, :, 1, :]
                    )
                    nc.vector.tensor_mul(
                        st["prod"][:, cs, 0:RU],
                        u_sb[:, :, 0, :],
                        u_sb[:, :, 1, :],
                    )
                else:
                    nc.vector.tensor_mul(
                        u_sb[:], tv[:, :, :, 0, :], tv[:, :, :, 1, :]
                    )
                    nc.vector.tensor_mul(
                        st["prod"][:, cs, 0:RU],
                        u_sb[:, :, 0, :],
                        u_sb[:, :, 1, :],
                    )
                if not RANKSUM_PE:
                    nc.vector.tensor_reduce(
                        st["rr"][:, cs],
                        st["prod"][:, cs, 0:RU].rearrange(
                            "p c (u r) -> p c u r", r=R
                        ),
                        AX.X,
                        OP.add,
                    )

            def xpose_prod(mi, st):
                if RANKSUM_PE:
                    # transpose prod -> [ru, c, b] (rank-sum runs next stage)
                    pt = qpool.tile([128, CHUNK, TILE_B], BF16, tag="pt")
                    nc.sync.dma_start_transpose(
                        pt[:], st["prod"][:].rearrange("p c f -> p (c f)")
                    )
                    st["pt"] = pt

            def last(mi, st):
                s_t = st["s_t"]
                o_sb = opool.tile([TILE_B, CHUNK, U], F32, tag="o")
                if RANKSUM_PE:
                    pt = st["pt"]
                    o_ps = ops_.tile([TILE_B, CHUNK, U], F32, tag="o_ps")
                    for c in range(CHUNK):
                        nc.tensor.matmul(
                            o_ps[:, c, :],
                            pt[0:RU, c, :],
                            ri_sb[:],
                            start=True,
                            stop=True,
                        )
                    rr = o_ps
                else:
                    rr = st["rr"]
                nc.gpsimd.tensor_tensor(
                    o_sb[:],
                    rr[:],
                    s_t[:].unsqueeze(2).broadcast_to([TILE_B, CHUNK, U]),
                    OP.mult,
                )
                nc.sync.dma_start(out=out[mi], in_=o_sb[:])

            DEPTH = 3  # front -> matmul/pair stage lookahead (macros)
            pend = {}
            for mi in range(N_MACRO + DEPTH + 2):
                if mi >= DEPTH + 2:
                    last(mi - DEPTH - 2, pend.pop(mi - DEPTH - 2))
                if mi >= DEPTH and mi - DEPTH < N_MACRO:
                    pairs_stage(mi - DEPTH, pend[mi - DEPTH])
                if mi >= DEPTH + 1 and mi - DEPTH - 1 < N_MACRO:
                    xpose_prod(mi - DEPTH - 1, pend[mi - DEPTH - 1])
                if mi < N_MACRO:
                    pend[mi] = front(mi)
                if mi >= DEPTH and mi - DEPTH < N_MACRO:
                    dve_pairs(mi - DEPTH, pend[mi - DEPTH])
                    tree_stage(mi - DEPTH, pend[mi - DEPTH], 0)
                    tree_stage(mi - DEPTH, pend[mi - DEPTH], 1)
    nc.finalize()
    return nc


def _pack_weights(kernel: np.ndarray):
    K = kernel.astype(np.float64)  # [2, R, F, U]
    C2 = np.zeros((128, 8 * RU), np.float32)
    for g in range(NG):
        kk, vv, side = g // 4, (g % 4) // 2, g % 2
        r0 = 64 * kk + 16 * (g % 4)
        c0 = 2 * RU * vv + RU * side
        fs = [4 * g, 4 * g + 1, 4 * g + 2, 4 * g + 3]
        for m in range(16):
            ab, cd = m % 4, m // 4
            bits = (ab & 1, (ab >> 1) & 1, cd & 1, (cd >> 1) & 1)
            coef = (
                K[bits[0], :, fs[0], :]
                * K[bits[1], :, fs[1], :]
                * K[bits[2], :, fs[2], :]
                * K[bits[3], :, fs[3], :]
            )  # [R, U]
            # ru columns u-major: col = u*R + r
            C2[r0 + m, c0 : c0 + RU] = coef.T.reshape(RU)
    rind = np.zeros((RU, U), np.float32)
    for u in range(U):
        rind[u * R : (u + 1) * R, u] = 1.0
    return C2.astype(np.float16), rind


_NC_CACHE = {}


def kernel(X: np.ndarray, kernel: np.ndarray) -> np.ndarray:
    if "nc" not in _NC_CACHE:
        _NC_CACHE["nc"] = build_nc()
    nc = _NC_CACHE["nc"]
    C2, rind = _pack_weights(kernel)
    X = np.ascontiguousarray(X, dtype=np.float32)
    # [core, macro, chunk, partition, F] -> [core, macro, partition, chunk, F]
    Xd = (
        X.reshape(N_CORES, N_MACRO, CHUNK, TILE_B, F)
        .transpose(0, 1, 3, 2, 4)
        .copy()
    )
    in_maps = []
    for c in range(N_CORES):
        in_maps.append(
            {"X": Xd[c], "C2": C2, "RIND": rind.astype(ml_dtypes.bfloat16)}
        )
    res = run_bass_kernel_spmd(nc, in_maps, core_ids=list(range(N_CORES)))
    outs = []
    for c in range(N_CORES):
        o = res.results[c]["out"]  # [N_MACRO, TILE_B, CHUNK, U]
        outs.append(o.transpose(0, 2, 1, 3).reshape(B_CORE, U))
    return np.concatenate(outs, axis=0).astype(np.float32)


if __name__ == "__main__":
    rng = np.random.default_rng(0)
    X = rng.standard_normal((B_FULL, F), dtype=np.float32)
    K = (rng.standard_normal((2, R, F, U)) * 0.24).astype(np.float32)
    y = kernel(X, K)
    print(y.shape, y.dtype, np.abs(y).max())
